# revision 3
# baseline (speedup 1.0000x reference)
"""AFM (Attentional Factorization Machine) distributed kernel for 8 Trainium2
NeuronCores.

Strategy (per sharding hint): data-parallel over the batch dimension.
Each of the 8 cores gets B/8 = 256 samples; the embedding table and the
small attention parameters are replicated to every core. All heavy
compute (embedding gather, pairwise interactions, attention MLP,
softmax, pooling) runs on-device; the host only slices the batch and
concatenates the 8 per-core outputs.

Hardcoded problem shape: B=2048, F=39, D=64, A=64, vocab=100000.
"""

import numpy as np

N_CORES = 8
B = 2048
F = 39
D = 64
A = 64
VOCAB = 100000

_ROW, _COL = np.triu_indices(F, k=1)  # P = 741 pairs


def _per_core_fn():
    import jax
    import jax.numpy as jnp

    row = jnp.asarray(_ROW, dtype=jnp.int32)
    col = jnp.asarray(_COL, dtype=jnp.int32)

    P = row.shape[0]

    def per_core(feat_index, feat_value, fo_w, emb_table, att_W, att_b,
                 att_h, p_vec, bias):
        b = feat_index.shape[0]
        fv = feat_value[:, :, None]                              # [b,F,1]
        # first-order term
        y_first = jnp.sum(fo_w[feat_index] * fv, axis=1)[:, 0]   # [b]
        # embeddings scaled by values
        feat_emb = emb_table[feat_index] * fv                    # [b,F,D]
        inter = feat_emb[:, row] * feat_emb[:, col]              # [b,P,D]
        # location-based attention: one flat matmul instead of einsum
        g = jnp.maximum(
            inter.reshape(b * P, D) @ att_W + att_b, 0.0)        # [bP,A]
        s = (g @ att_h[:, None]).reshape(b, P)                   # [b,P]
        w = jnp.exp(s - jnp.max(s, axis=1, keepdims=True))       # [b,P]
        num = jnp.einsum('bp,bpd->bd', w, inter)                 # [b,D]
        att_pool = (num @ p_vec) / jnp.sum(w, axis=1)            # [b]
        return jax.nn.sigmoid(bias + y_first + att_pool)         # [b]

    return per_core


def kernel(feat_index, feat_value, fo_w, emb_table, att_W, att_b, att_h,
           p_vec, bias):
    import jax

    out_dtype = np.float32

    feat_index = np.asarray(feat_index)
    if feat_index.dtype not in (np.int32,):
        feat_index = feat_index.astype(np.int32)
    feat_value = np.asarray(feat_value, dtype=np.float32)
    fo_w = np.asarray(fo_w, dtype=np.float32)
    emb_table = np.asarray(emb_table, dtype=np.float32)
    att_W = np.asarray(att_W, dtype=np.float32)
    att_b = np.asarray(att_b, dtype=np.float32)
    att_h = np.asarray(att_h, dtype=np.float32)
    p_vec = np.asarray(p_vec, dtype=np.float32)
    bias = np.asarray(bias, dtype=np.float32)

    devices = jax.devices()[:N_CORES]
    assert len(devices) == N_CORES, f"need {N_CORES} cores, got {len(devices)}"

    per_core = _per_core_fn()
    # pmap: one compile, replicated to all 8 cores; batch sharded on axis 0,
    # params broadcast.
    pmapped = jax.pmap(
        per_core,
        in_axes=(0, 0, None, None, None, None, None, None, None),
        devices=devices,
    )

    bs = feat_index.shape[0] // N_CORES
    fi_sh = feat_index.reshape(N_CORES, bs, F)
    fv_sh = feat_value.reshape(N_CORES, bs, F)

    out = pmapped(fi_sh, fv_sh, fo_w, emb_table, att_W, att_b, att_h,
                  p_vec, bias)
    return np.asarray(out, dtype=out_dtype).reshape(-1)


if __name__ == "__main__":
    rng = np.random.default_rng(0)
    fi = rng.integers(0, VOCAB, size=(B, F)).astype(np.int32)
    fv = rng.random((B, F), dtype=np.float32)
    out = kernel(
        feat_index=fi, feat_value=fv,
        fo_w=rng.standard_normal((VOCAB, 1), dtype=np.float32) * 0.01,
        emb_table=rng.standard_normal((VOCAB, D), dtype=np.float32) * 0.01,
        att_W=rng.standard_normal((D, A), dtype=np.float32) * 0.125,
        att_b=rng.standard_normal((A,), dtype=np.float32),
        att_h=rng.standard_normal((A,), dtype=np.float32),
        p_vec=rng.standard_normal((D,), dtype=np.float32),
        bias=np.float32(0.1),
    )
    print(out.shape, out.dtype, out[:4])
